# revision 16
# baseline (speedup 1.0000x reference)
"""Causal self-attention (GQA + RoPE) Trainium2 kernel over 8 NeuronCores.

Sharding: 8 cores = batch(2) x kv_head(4). Each core computes its batch's
4 q-heads / 1 kv-head attention plus the partial output projection; host
sums the 4 partial projections per batch element.

Device algorithm (fully transposed "k-major" attention, zero P-transposes):
  one unified PSUM pool set for the whole program (psS 4 banks + psY 2 +
    aux 2 = 8) so there are no pool-boundary barriers between phases
  stage A: KV^T projection cc-major (paced by the x DMA stream); K^T roped
    on DVE straight out of PSUM and row-duplicated via a DVE copy; V rows
    staged bf16 by the scalar engine and transposed on the PE into t-major
    vhat tiles (ones column appended so the PV matmul also emits softmax
    sums); Q0/Q1 projections per-512-chunk, staged PSUM->SBUF by the scalar
    engine so the DVE rope backlog never blocks PSUM buffer rotation
  cos/sin tables ship as [64,T] bf16 (interleaved-pair rows, 8x smaller)
    and are row-duplicated on device
  attention in two head-pair passes; S^T = K^T.T @ Q^T half-array matmuls
    alternate row groups so pairs overlap in the PE; one exp ACTIVATE per
    two S tiles; causal staircase trimming; tri-mask multiply per diagonal
    block; normalize via fast reciprocal read directly from PSUM + gpsimd
    partition_broadcast
  Q1 projection + its rope interleave into pass 0 as PE filler; the output
    projection interleaves into pass 1; both passes run chunks (3,2,1,0)
    so the last chunk (and its trailing projection) is the smallest
  partial f32 out to HBM per 512-column half as soon as it is ready
"""

import sys

sys.path.insert(0, "/opt/trn_rl_repo")

import numpy as np
import ml_dtypes

import concourse.bacc as bacc
import concourse.tile as tile
from concourse import mybir
from concourse.bass_utils import run_bass_kernel_spmd

F32 = mybir.dt.float32
BF16 = mybir.dt.bfloat16
AF = mybir.ActivationFunctionType

T, C, D, H, HKV = 2048, 1024, 64, 16, 4
G = H // HKV  # q heads per kv head
NCC = C // 128  # 8 contraction chunks
NJQ = 4  # tq chunks of 512
TQC = 512
NTK = T // 128  # 16 tk tiles
SCALE = 1.0 / 8.0  # 1/sqrt(D)
NWU = 10  # warmup matmuls (HAM clock ramp during the DMA window)
JQ_ORDER = (3, 2, 1, 0)

_PROG = {}


def _build_program():
    nc = bacc.Bacc()
    xT_d = nc.dram_tensor("xT", [C, T], BF16, kind="ExternalInput")
    w_d = nc.dram_tensor("w_all", [C, 384], BF16, kind="ExternalInput")
    wp_d = nc.dram_tensor("wp", [256, C], BF16, kind="ExternalInput")
    cq_d = nc.dram_tensor("cos_q", [64, T], F32, kind="ExternalInput")
    sq_d = nc.dram_tensor("sin_q", [64, T], F32, kind="ExternalInput")
    mk_d = nc.dram_tensor("masks", [128, 128], BF16, kind="ExternalInput")
    id_d = nc.dram_tensor("identb", [128, 128], BF16, kind="ExternalInput")
    out_d = nc.dram_tensor("out_p", [T, C], F32, kind="ExternalOutput")

    swap_mask = [i ^ 1 for i in range(32)]

    with tile.TileContext(nc) as tc:
        with (
            tc.tile_pool(name="const", bufs=1) as const,
            tc.tile_pool(name="big", bufs=1) as big,
            tc.tile_pool(name="xp", bufs=1) as xp,
            tc.tile_pool(name="rope", bufs=2) as ropep,
            tc.tile_pool(name="ptiles", bufs=8) as ppool,
            tc.tile_pool(name="small", bufs=3) as small,
            tc.tile_pool(name="outp", bufs=4) as outp,
            tc.tile_pool(name="psS", bufs=1, space="PSUM") as psS,
            tc.tile_pool(name="psY", bufs=2, space="PSUM") as psY,
            tc.tile_pool(name="aux", bufs=2, space="PSUM") as aux,
        ):
            W_sb = const.tile([128, NCC, 384], BF16, tag="W", name="W_sb")
            cq_sb = const.tile([128, T], F32, tag="cq", name="cq_sb")
            sq_sb = const.tile([128, T], F32, tag="sq", name="sq_sb")
            mk_sb = const.tile([128, 128], BF16, tag="mk", name="mk_sb")
            id_sb = const.tile([128, 128], BF16, tag="idb", name="id_sb")
            wp_sb = const.tile([128, 2, C], BF16, tag="wp", name="wp_sb")

            qrope = [big.tile([128, T], BF16, tag=f"qr{m}", name=f"qr{m}") for m in range(2)]
            k2 = big.tile([128, T], BF16, tag="k2", name="k2")
            vstage = big.tile([128, T], BF16, tag="vst", name="vstage")
            vhat = big.tile([128, NTK, 65], BF16, tag="vhat", name="vhat")
            yn = [big.tile([128, T], BF16, tag=f"yn{m}", name=f"yn{m}") for m in range(2)]

            # ---- input DMAs; cos/sin mid-stream so they land before rope ----
            xts = []
            for cc in range(NCC):
                xt = xp.tile([128, T], BF16, tag=f"x{cc}", name=f"x{cc}")
                nc.sync.dma_start(out=xt[:], in_=xT_d[cc * 128 : (cc + 1) * 128, :])
                nc.sync.dma_start(out=W_sb[:, cc, :], in_=w_d[cc * 128 : (cc + 1) * 128, :])
                xts.append(xt)
                if cc == 2:
                    nc.sync.dma_start(out=cq_sb[0:64, :], in_=cq_d[:])
                    nc.sync.dma_start(out=sq_sb[0:64, :], in_=sq_d[:])
            nc.sync.dma_start(out=mk_sb[:], in_=mk_d[:])
            nc.sync.dma_start(out=id_sb[:], in_=id_d[:])
            nc.sync.dma_start(out=wp_sb[:], in_=wp_d[:].rearrange("(n p) m -> p n m", p=128))

            # ---- early DVE work: warmup seed, vhat ones, cos/sin row-dup ----
            wu = ropep.tile([128, 512], BF16, tag="wu", name="wu", bufs=1)
            nc.vector.memset(wu[:], 0.5)
            nc.vector.memset(vhat[:, :, 64:65], 1.0)
            # row duplication crosses partitions -> must go through DMA
            nc.sync.dma_start(out=cq_sb[64:128, :], in_=cq_sb[0:64, :])
            nc.sync.dma_start(out=sq_sb[64:128, :], in_=sq_sb[0:64, :])

            def emit_rope_chunk(dst, src, jq, rows):
                """RoPE one 512-col chunk; src[:rows, 512] may be PSUM or SBUF;
                dst gets bf16 rows [0:rows] at cols [jq*TQC:(jq+1)*TQC]."""
                cs = slice(jq * TQC, (jq + 1) * TQC)
                shuf = ropep.tile([128, TQC], F32, tag="shuf", name="shuf")
                prod = ropep.tile([128, TQC], F32, tag="prod", name="prod")
                nc.vector.stream_shuffle(shuf[:rows, :], src[:rows, :], mask=swap_mask)
                nc.vector.tensor_mul(out=shuf[:rows, :], in0=shuf[:rows, :], in1=sq_sb[:rows, cs])
                nc.vector.tensor_mul(out=prod[:rows, :], in0=src[:rows, :], in1=cq_sb[:rows, cs])
                nc.vector.tensor_add(out=dst[0:rows, cs], in0=prod[:rows, :], in1=shuf[:rows, :])

            # ---- warmup: garbage matmuls ramp HAM while the x DMA streams ----
            puw = aux.tile([128, TQC], F32, tag="ax", name="puw")
            for _ in range(NWU):
                nc.tensor.matmul(puw[:], lhsT=wu[:, 0:128], rhs=wu[:], start=True, stop=True)

            # ---- stage A: KV^T projection, cc-major (follows the DMA) ----
            A2 = psS.tile([128, 2, TQC], F32, tag="sg0", name="A2")
            B2 = psS.tile([128, 2, TQC], F32, tag="sg1", name="B2")
            kvt = [A2, A2, B2, B2]
            for jq in range(NJQ):
                for cc in range(NCC):
                    nc.tensor.matmul(
                        kvt[jq][:, jq % 2, :],
                        lhsT=W_sb[:, cc, 256:384],
                        rhs=xts[cc][:, jq * TQC : (jq + 1) * TQC],
                        start=(cc == 0),
                        stop=(cc == NCC - 1),
                    )
            # K rope from PSUM (chunk order 0..3: attention consumes ik
            # ascending), then duplicate into rows 64:128 for the hh=1 row
            # group; V rows staged bf16 via the scalar engine
            for jq in range(NJQ):
                cs = slice(jq * TQC, (jq + 1) * TQC)
                src = kvt[jq][:, jq % 2, :]
                kst = small.tile([128, TQC], F32, tag="qst", name="kst")
                nc.scalar.copy(out=kst[0:64, :], in_=src[0:64, :])
                emit_rope_chunk(k2, kst, jq, rows=64)
                nc.sync.dma_start(out=k2[64:128, cs], in_=k2[0:64, cs])
                nc.scalar.copy(out=vstage[64:128, cs], in_=src[64:128, :])

            # ---- stage A: Q0 per-chunk projection; scalar stages PSUM->SBUF
            #      so DVE rope never blocks aux rotation; V transposes
            #      (tt ascending) interleaved as PE work ----
            def q_chunk(pa, dst, jq):
                qst = small.tile([128, TQC], F32, tag="qst", name="qst")
                nc.scalar.copy(out=qst[:], in_=pa[:])
                emit_rope_chunk(dst, qst, jq, rows=128)

            for idx, jq in enumerate(JQ_ORDER):
                pa = aux.tile([128, TQC], F32, tag="ax", name=f"pa0_{jq}")
                for cc in range(NCC):
                    nc.tensor.matmul(
                        pa[:],
                        lhsT=W_sb[:, cc, 0:128],
                        rhs=xts[cc][:, jq * TQC : (jq + 1) * TQC],
                        start=(cc == 0),
                        stop=(cc == NCC - 1),
                    )
                q_chunk(pa, qrope[0], jq)
                for tt in range(4 * idx, 4 * idx + 4):
                    pt_ = psY.tile([128, 64], BF16, tag="py", name="ptr")
                    nc.tensor.transpose(
                        pt_[:],
                        vstage[64:128, tt * 128 : (tt + 1) * 128],
                        id_sb[64:128, 0:64],
                    )
                    nc.scalar.copy(out=vhat[:, tt, 0:64], in_=pt_[:])

            # ---- Q1 projection generator: pass-0 PE filler ----
            def a2_gen():
                for jq2 in JQ_ORDER:
                    pa = aux.tile([128, TQC], F32, tag="ax", name=f"pa1_{jq2}")
                    for cc in range(NCC):
                        nc.tensor.matmul(
                            pa[:],
                            lhsT=W_sb[:, cc, 128:256],
                            rhs=xts[cc][:, jq2 * TQC : (jq2 + 1) * TQC],
                            start=(cc == 0),
                            stop=(cc == NCC - 1),
                        )
                        yield
                    q_chunk(pa, qrope[1], jq2)
                yield

            gen = a2_gen()

            def filler0():
                next(gen, None)
                next(gen, None)

            def attention_pass(hp, filler, boundary):
                """One head-pair pass over chunks in JQ_ORDER. filler() emits
                extra PE work per ik-pair; boundary(i) runs after chunk i
                (position in JQ_ORDER) before its normalize."""
                for i, jq in enumerate(JQ_ORDER):
                    nik = 4 * jq + 4
                    pys = [psY.tile([65, TQC], F32, tag="py", name="py") for _ in range(2)]
                    for ika in range(0, nik, 2):
                        iks = (ika, ika + 1)
                        filler()
                        qt = qrope[hp]
                        los = [max(ik - 4 * jq, 0) * 128 for ik in iks]
                        ps_gs = [
                            psS.tile([128, 2, TQC], F32, tag=f"sg{hh}", name=f"ps_g{hh}")
                            for hh in range(2)
                        ]
                        # alternate row groups (hh base 0 / 64) so consecutive
                        # half-array S matmuls overlap in the PE array
                        for gi, ik in enumerate(iks):
                            lo = los[gi]
                            for hh in range(2):
                                base = hh * 64
                                nc.tensor.matmul(
                                    ps_gs[hh][:, gi, lo:TQC],
                                    lhsT=k2[base : base + 64, ik * 128 : (ik + 1) * 128],
                                    rhs=qt[base : base + 64, jq * TQC + lo : (jq + 1) * TQC],
                                    start=True,
                                    stop=True,
                                )
                        mlo = min(los)
                        ptiles = []
                        for hh in range(2):
                            ptile = ppool.tile([128, 2, TQC], BF16, tag="pt", name="ptile")
                            nc.scalar.activation(
                                out=ptile[:, :, mlo:TQC],
                                in_=ps_gs[hh][:, :, mlo:TQC],
                                func=AF.Exp,
                                scale=SCALE,
                            )
                            ptiles.append(ptile)
                        for hh in range(2):
                            for gi, ik in enumerate(iks):
                                if ik - 4 * jq >= 0:
                                    lo = los[gi]
                                    nc.vector.tensor_mul(
                                        out=ptiles[hh][:, gi, lo : lo + 128],
                                        in0=ptiles[hh][:, gi, lo : lo + 128],
                                        in1=mk_sb[:, 0:128],
                                    )
                        for hh in range(2):
                            for gi, ik in enumerate(iks):
                                lo = los[gi]
                                nc.tensor.matmul(
                                    pys[hh][:, lo:TQC],
                                    lhsT=vhat[:, ik, :],
                                    rhs=ptiles[hh][:, gi, lo:TQC],
                                    start=(ik == 0),
                                    stop=(ik == nik - 1),
                                )
                    boundary(i)
                    cs = slice(jq * TQC, (jq + 1) * TQC)
                    for hh in range(2):
                        # eager copy frees the PSUM accumulator; reciprocal
                        # reads the sums row straight from PSUM
                        ybuf = small.tile([65, TQC], F32, tag="ybuf", name="ybuf")
                        nc.vector.tensor_copy(out=ybuf[:], in_=pys[hh][:])
                        srow = small.tile([1, TQC], F32, tag="srow", name="srow")
                        nc.vector.tensor_copy(out=srow[:], in_=pys[hh][64:65, :])
                        rinv = small.tile([1, TQC], F32, tag="rinv", name="rinv")
                        nc.vector.reciprocal_approx_fast(out=rinv[:], in_=srow[:])
                        rb = small.tile([64, TQC], F32, tag="rb", name="rb")
                        nc.gpsimd.partition_broadcast(rb[:], rinv[:])
                        nc.vector.tensor_mul(
                            out=yn[hp][hh * 64 : hh * 64 + 64, cs],
                            in0=ybuf[0:64, :],
                            in1=rb[:],
                        )

            # ---- pass 0 (heads 0,1); Q1 projection + its rope as filler ----
            for _ in range(10):
                filler0()
            attention_pass(0, filler0, lambda i: None)
            for _ in gen:
                pass

            # ---- pass 1 (heads 2,3) with output projection interleaved ----
            def emit_proj(pjq):
                for tt in range(4 * pjq, 4 * pjq + 4):
                    pps = [aux.tile([128, 512], F32, tag="ax", name=f"pp{n}") for n in range(2)]
                    for kk in range(2):
                        for ncol in range(2):
                            nc.tensor.matmul(
                                pps[ncol][:],
                                lhsT=yn[kk][:, tt * 128 : (tt + 1) * 128],
                                rhs=wp_sb[:, kk, ncol * 512 : (ncol + 1) * 512],
                                start=(kk == 0),
                                stop=(kk == 1),
                            )
                    for ncol in range(2):
                        outsb = outp.tile([128, 512], F32, tag="osb", name="osb")
                        nc.vector.tensor_copy(out=outsb[:], in_=pps[ncol][:])
                        nc.sync.dma_start(
                            out=out_d[tt * 128 : (tt + 1) * 128, ncol * 512 : (ncol + 1) * 512],
                            in_=outsb[:],
                        )

            def boundary1(i):
                if i > 0:
                    emit_proj(JQ_ORDER[i - 1])

            attention_pass(1, lambda: None, boundary1)
            emit_proj(JQ_ORDER[-1])

    nc.compile()
    return nc


def _host_tables():
    # RoPE tables in interleaved-pair device layout (row j'=2i <-> orig j=i,
    # j'=2i+1 <-> orig j=i+32); sign of the shuffled sin term folded in.
    # Only the 64 unique rows ship (bf16); the device duplicates rows 64:128.
    inv = 1.0 / (10000.0 ** (np.arange(0, D, 2, dtype=np.float64) / D))  # (32,)
    t = np.arange(T, dtype=np.float64)
    fr = np.outer(t, inv)  # (T, 32)
    cos_h = np.cos(fr).T.astype(np.float32)  # (32, T)
    sin_h = np.sin(fr).T.astype(np.float32)
    cosI = np.empty((D, T), np.float32)
    sinI = np.empty((D, T), np.float32)
    cosI[0::2] = cos_h
    cosI[1::2] = cos_h
    sinI[0::2] = -sin_h
    sinI[1::2] = sin_h
    # tri mask for the diagonal 128-block: allowed iff tkl <= tql
    tkl = np.arange(128)[:, None]
    tql = np.arange(128)[None, :]
    mask = (tkl <= tql).astype(np.float32).astype(ml_dtypes.bfloat16)
    identb = np.tile(np.eye(64, dtype=np.float32), (2, 2)).astype(ml_dtypes.bfloat16)
    return cosI, sinI, mask, identb


def make_in_maps(x, wq, wk, wv, wproj):
    cos_q, sin_q, mask, identb = _host_tables()
    # interleave permutation within each head's 64 cols: perm[2i]=i, perm[2i+1]=i+32
    perm = np.empty(D, np.int64)
    perm[0::2] = np.arange(32)
    perm[1::2] = np.arange(32) + 32
    in_maps = []
    for c in range(8):
        b, h = c // 4, c % 4
        xT = np.ascontiguousarray(x[b].T).astype(ml_dtypes.bfloat16)  # (C, T)
        wq_h = wq[:, h * 256 : (h + 1) * 256].reshape(C, G, D)[:, :, perm].reshape(C, 256)
        wk_h = wk[:, h * 64 : (h + 1) * 64][:, perm]
        wv_h = wv[:, h * 64 : (h + 1) * 64]
        w_all = np.concatenate([wq_h, wk_h, wv_h], axis=1).astype(ml_dtypes.bfloat16)
        wp_h = wproj[h * 256 : (h + 1) * 256, :].astype(ml_dtypes.bfloat16)
        in_maps.append(
            {
                "xT": xT,
                "w_all": w_all,
                "wp": wp_h,
                "cos_q": cos_q,
                "sin_q": sin_q,
                "masks": mask,
                "identb": identb,
            }
        )
    return in_maps


def kernel(x, wq, wk, wv, wproj):
    x = np.asarray(x, dtype=np.float32)
    wq = np.asarray(wq, dtype=np.float32)
    wk = np.asarray(wk, dtype=np.float32)
    wv = np.asarray(wv, dtype=np.float32)
    wproj = np.asarray(wproj, dtype=np.float32)
    B = x.shape[0]

    if "nc" not in _PROG:
        _PROG["nc"] = _build_program()
    nc = _PROG["nc"]

    in_maps = make_in_maps(x, wq, wk, wv, wproj)

    res = run_bass_kernel_spmd(nc, in_maps, list(range(8)))
    out = np.zeros((B, T, C), np.float32)
    for c in range(8):
        out[c // 4] += res.results[c]["out_p"]
    return out
